# revision 4
# baseline (speedup 1.0000x reference)
"""Bass/Trainium2 kernel for nn_Bert_coss (8-core data-parallel over batch).

Computation (per example):
  o1 = relu(X1 @ W.T + b)            [S, H]
  o2 = relu(X2 @ W.T + b)            [S, H]
  o1_doc, o2_doc = mean over S       [H]
  out = sigmoid(relu(concat(o1_doc, o2_doc) @ fd_w.T + fd_b) @ ff_w.T + ff_b)
  scores[s] = o1e[s] . o2_doc   (o1e = o1 ++ o1_doc row), s in 0..S
  att = softmax(scores); output rows 0..S-1 = att[0:S], row S = out.

The reference's full [S+1,S+1] co-attention einsum is only consumed through
its last column, so only S+1 dot products against o2_doc are needed.

Main-matmul strategy: fp8(E4M3) DoubleRow matmuls (K=256/instr, 0.5 cyc/row
— 4x fp16 PE throughput). Precision recovered with an error-compensated
3-product scheme on the score-critical o1 path:
    X1 = X8 + Xr8 (two fp8 planes), W*64 = W8 + Wr8 (two fp8 planes)
    64*o1_pre = X8@W8 + X8@Wr8 + Xr8@W8       (dropped Xr8@Wr8 ~ 1e-4 rel)
The o2 path runs a single product X28@W8: its per-entry noise is averaged
over the S=512 doc-mean and the shared component cancels in the softmax
(validated: end-to-end rel err 1.9e-3 vs the 2e-2 gate).

Layout: host packs, per example, [128 partitions, 3 planes x 6 k-chunks x
512 positions] fp8 so each example is ONE contiguous DMA. The relu+bias
(+1/64 scale) evictions ride ACT for o1 (accum_out = doc sums) and the
Vector engine for o2 (tensor_scalar add-bias/max-0 with accum_out), keeping
both engines under the DMA-bound critical path. Scores stay fp16 matvecs.
"""

import sys

for _p in ("/opt/trn_rl_repo",):
    if _p not in sys.path:
        sys.path.append(_p)

import numpy as np
import ml_dtypes
from contextlib import ExitStack

import concourse.bass as bass
import concourse.tile as tile
from concourse import bacc, mybir
from concourse import bass_utils

B, S, V, H = 64, 512, 768, 256
NCORES = 8
BL = B // NCORES        # examples per core
KV = V // 128           # contraction chunks for the mlp matmul
KP = KV // 2            # DoubleRow k-pairs
MH = H // 128           # output-partition chunks of H
WSCALE = 64.0           # W pre-scale so fp8 e4m3 covers its range

F32 = mybir.dt.float32
F16 = mybir.dt.float16
F8 = mybir.dt.float8e4
AF = mybir.ActivationFunctionType
DR = mybir.MatmulPerfMode.DoubleRow
E4NP = ml_dtypes.float8_e4m3


def _build_kernel(tc):
    nc = tc.nc
    x_all = nc.dram_tensor("x_all", [BL, 128, 3 * KV * S], F8,
                           kind="ExternalInput").ap()
    w_all = nc.dram_tensor("w_all", [128, 2 * KV * H], F8,
                           kind="ExternalInput").ap()
    mlpb_d = nc.dram_tensor("mlpb", [128, MH], F32, kind="ExternalInput").ap()
    mlpb64_d = nc.dram_tensor("mlpb64", [128, MH], F32,
                              kind="ExternalInput").ap()
    fdw_d = nc.dram_tensor("fdw", [128, 4 * H], F16, kind="ExternalInput").ap()
    fdb_d = nc.dram_tensor("fdb", [128, MH], F32, kind="ExternalInput").ap()
    ffw_d = nc.dram_tensor("ffw", [128, MH], F16, kind="ExternalInput").ap()
    nffb_d = nc.dram_tensor("nffb", [1, 1], F32, kind="ExternalInput").ap()
    out = nc.dram_tensor("out", [BL, S + 1], F32, kind="ExternalOutput").ap()

    with ExitStack() as ctx:
        const = ctx.enter_context(tc.tile_pool(name="const", bufs=1))

        w8 = const.tile([128, 2 * KV * H], F8)
        nc.scalar.dma_start(w8[:], w_all)
        w8_v = w8[:].rearrange("p (w k h) -> p w k h", w=2, k=KV)
        mlpb_sb = const.tile([128, MH], F32)
        mlpb64_sb = const.tile([128, MH], F32)
        nc.scalar.dma_start(mlpb_sb[:], mlpb_d)
        nc.scalar.dma_start(mlpb64_sb[:], mlpb64_d)
        fdw_sb = const.tile([128, 4 * H], F16)
        fdb_sb = const.tile([128, MH], F32)
        ffw_sb = const.tile([128, MH], F16)
        nffb_sb = const.tile([1, 1], F32)
        # dummy Exp so the ACT table set loads during the DMA ramp instead of
        # on the end-of-kernel critical path
        expwarm = const.tile([1, 1], F32)
        nc.scalar.activation(expwarm[:], mlpb_sb[0:1, 0:1], AF.Exp, scale=0.0)

        def _late_const_dmas():
            # parameters only needed by the end-of-kernel head
            nc.scalar.dma_start(fdw_sb[:], fdw_d)
            nc.scalar.dma_start(fdb_sb[:], fdb_d)
            nc.scalar.dma_start(ffw_sb[:], ffw_d)
            nc.scalar.dma_start(nffb_sb[:], nffb_d)

        # doc-vector raw sums; column b*4 + c, c in (o1m0, o1m1, o2m0, o2m1)
        # o1 columns hold 512*o1_doc; o2 columns hold 512*64*o2_doc
        docs_all = const.tile([128, 4 * BL], F32)
        # true-scale doc vectors in fp16 (score matvec lhsT + head rhs)
        dscs = const.tile([128, 4 * BL], F16)

        with ExitStack() as mctx:
            xpool = mctx.enter_context(tc.tile_pool(name="x", bufs=3))
            o1pool = mctx.enter_context(tc.tile_pool(name="o1", bufs=2))
            o2pool = mctx.enter_context(tc.tile_pool(name="o2", bufs=2))
            apool = mctx.enter_context(tc.tile_pool(name="att", bufs=3))
            mm1_ps = mctx.enter_context(tc.tile_pool(name="mm1", bufs=2, space="PSUM"))
            mm2_ps = mctx.enter_context(tc.tile_pool(name="mm2", bufs=1, space="PSUM"))
            sc_ps = mctx.enter_context(tc.tile_pool(name="scps", bufs=1, space="PSUM"))
            dd_ps = mctx.enter_context(tc.tile_pool(name="ddps", bufs=1, space="PSUM"))

            def do_scores(b, o1T):
                ssc = sc_ps.tile([1, S], F32)
                for hk in range(MH):
                    nc.tensor.matmul(
                        ssc[:],
                        dscs[:, b * 4 + 2 + hk : b * 4 + 3 + hk],
                        o1T[:, hk * S : (hk + 1) * S],
                        start=(hk == 0),
                        stop=(hk == MH - 1),
                    )
                sdd = dd_ps.tile([1, 1], F32)
                for hk in range(MH):
                    nc.tensor.matmul(
                        sdd[:],
                        dscs[:, b * 4 + 2 + hk : b * 4 + 3 + hk],
                        dscs[:, b * 4 + hk : b * 4 + hk + 1],
                        start=(hk == 0),
                        stop=(hk == MH - 1),
                    )
                # softmax on partition 0, straight from PSUM; no max-
                # subtraction (scores are O(25), far inside fp32 exp range)
                att = apool.tile([1, S], F32)
                s1 = apool.tile([1, 1], F32, name="s1")
                nc.scalar.activation(att[:], ssc[:], AF.Exp, accum_out=s1[:])
                edd = apool.tile([1, 1], F32, name="edd")
                nc.scalar.activation(edd[:], sdd[:], AF.Exp)
                stot = apool.tile([1, 1], F32, name="stot")
                nc.vector.tensor_add(stot[:], s1[:], edd[:])
                rs = apool.tile([1, 1], F32, name="rs")
                nc.vector.reciprocal(rs[:], stot[:])
                nc.vector.tensor_scalar_mul(att[:], att[:], rs[:])
                # SWDGE: keeps the blocking wait off the ACT/SP sequencers
                nc.gpsimd.dma_start(out[b : b + 1, 0:S], att[:])

            # o1 = (X8@W8 + X8@Wr8 + Xr8@W8) / 64 ; plane 2 (X2) uses W8 only
            PRODS1 = ((0, 0), (0, 1), (1, 0))
            prev = None
            for b in range(BL):
                xt = xpool.tile([128, 3 * KV * S], F8, tag="xt", name="xt")
                xt_v = xt[:].rearrange("p (c k s) -> p c k s", c=3, k=KV)
                if b == 0:
                    # first example streams per plane so the PE can start on
                    # plane 0 while planes 1-2 are still in flight
                    xsrc = x_all[b].rearrange("p (c k s) -> p c k s", c=3, k=KV)
                    for c in range(3):
                        nc.sync.dma_start(xt_v[:, c, :, :], xsrc[:, c, :, :])
                else:
                    nc.sync.dma_start(xt[:], x_all[b])
                if b == 1:
                    _late_const_dmas()

                o1T = o1pool.tile([128, MH * S], F16)
                pss1 = [
                    mm1_ps.tile([128, S], F32, tag=f"p1{m}", name=f"p1{m}")
                    for m in range(MH)
                ]
                for pi, (xp, wp) in enumerate(PRODS1):
                    for j in range(KP):
                        for m in range(MH):
                            nc.tensor.matmul(
                                pss1[m][:],
                                w8_v[:, wp, 2 * j : 2 * j + 2,
                                     m * 128 : (m + 1) * 128],
                                xt_v[:, xp, 2 * j : 2 * j + 2, :],
                                start=(pi == 0 and j == 0),
                                stop=(pi == len(PRODS1) - 1 and j == KP - 1),
                                perf_mode=DR,
                            )
                # scores for the previous example slot in here: their inputs
                # (o1T, dscs) are ready by now, so the PE never stalls on them
                if prev is not None:
                    do_scores(*prev)
                pss2 = [
                    mm2_ps.tile([128, S], F32, tag=f"p2{m}", name=f"p2{m}")
                    for m in range(MH)
                ]
                for j in range(KP):
                    for m in range(MH):
                        nc.tensor.matmul(
                            pss2[m][:],
                            w8_v[:, 0, 2 * j : 2 * j + 2,
                                 m * 128 : (m + 1) * 128],
                            xt_v[:, 2, 2 * j : 2 * j + 2, :],
                            start=(j == 0),
                            stop=(j == KP - 1),
                            perf_mode=DR,
                        )

                for m in range(MH):
                    nc.scalar.activation(
                        o1T[:, m * S : (m + 1) * S],
                        pss1[m][:],
                        AF.Relu,
                        bias=mlpb_sb[:, m : m + 1],
                        scale=1.0 / WSCALE,
                        accum_out=docs_all[:, b * 4 + m : b * 4 + m + 1],
                    )
                for m in range(MH):
                    # relu(p/64 + b) * 64 = max(p + 64b, 0); the 64 folds into
                    # the dscs scaling below. Only the doc sum is consumed.
                    o2scr = o2pool.tile([128, S], F16, tag="o2scr", name="o2scr")
                    nc.vector.tensor_scalar(
                        o2scr[:],
                        pss2[m][:],
                        mlpb64_sb[:, m : m + 1],
                        0.0,
                        mybir.AluOpType.add,
                        mybir.AluOpType.max,
                    )
                    nc.vector.tensor_reduce(
                        docs_all[:, b * 4 + 2 + m : b * 4 + 3 + m],
                        o2scr[:],
                        mybir.AxisListType.X,
                        mybir.AluOpType.add,
                    )
                nc.vector.tensor_scalar_mul(
                    dscs[:, b * 4 : b * 4 + 2],
                    docs_all[:, b * 4 : b * 4 + 2], 1.0 / S)
                nc.vector.tensor_scalar_mul(
                    dscs[:, b * 4 + 2 : b * 4 + 4],
                    docs_all[:, b * 4 + 2 : b * 4 + 4], 1.0 / (S * WSCALE))
                prev = (b, o1T)
            do_scores(*prev)

        # ---- head (batched over the BL examples) ----
        with ExitStack() as hctx:
            hpool = hctx.enter_context(tc.tile_pool(name="head", bufs=2))
            h_ps = hctx.enter_context(tc.tile_pool(name="hps", bufs=2, space="PSUM"))
            o_ps = hctx.enter_context(tc.tile_pool(name="ops", bufs=1, space="PSUM"))

            docs_v = dscs[:].rearrange("p (b k) -> p k b", k=4)
            fdw_v = fdw_sb[:].rearrange("p (k h) -> p k h", k=4)
            h16 = hpool.tile([128, MH * BL], F16)
            for m in range(MH):
                ph = h_ps.tile([128, BL], F32)
                for kc in range(4):
                    nc.tensor.matmul(
                        ph[:],
                        fdw_v[:, kc, m * 128 : (m + 1) * 128],
                        docs_v[:, kc, :],
                        start=(kc == 0),
                        stop=(kc == 3),
                    )
                nc.scalar.activation(
                    h16[:, m * BL : (m + 1) * BL],
                    ph[:],
                    AF.Relu,
                    bias=fdb_sb[:, m : m + 1],
                )
            po = o_ps.tile([1, BL], F32)
            for m in range(MH):
                nc.tensor.matmul(
                    po[:],
                    ffw_sb[:, m : m + 1],
                    h16[:, m * BL : (m + 1) * BL],
                    start=(m == 0),
                    stop=(m == MH - 1),
                )
            # sigmoid(x) = 1/(1+exp(-x)) — stays in the Exp table set
            sig_row = hpool.tile([1, BL], F32)
            nc.scalar.activation(sig_row[:], po[:], AF.Exp,
                                 bias=nffb_sb[0:1, 0:1], scale=-1.0)
            nc.vector.tensor_scalar_add(sig_row[:], sig_row[:], 1.0)
            nc.vector.reciprocal(sig_row[:], sig_row[:])

            # final output column: out[:, S] = sigmoid head values
            nc.gpsimd.dma_start(
                out[:, S : S + 1],
                sig_row[0:1, :].rearrange("o (b s) -> o b s", b=BL),
            )


_NC_CACHE = None


def _get_nc():
    global _NC_CACHE
    if _NC_CACHE is None:
        nc = bacc.Bacc("TRN2", target_bir_lowering=False, debug=False,
                       num_devices=NCORES)
        with tile.TileContext(nc) as tc:
            _build_kernel(tc)
        nc.compile()
        _NC_CACHE = nc
    return _NC_CACHE


def _q8(a):
    return np.ascontiguousarray(a).astype(E4NP)


def kernel(output_1, output_2, mlp_w, mlp_b, fd_w, fd_b, ff_w, ff_b):
    output_1 = np.asarray(output_1, dtype=np.float32)
    output_2 = np.asarray(output_2, dtype=np.float32)
    mlp_w = np.asarray(mlp_w, dtype=np.float32)
    mlp_b = np.asarray(mlp_b, dtype=np.float32)
    fd_w = np.asarray(fd_w, dtype=np.float32)
    fd_b = np.asarray(fd_b, dtype=np.float32)
    ff_w = np.asarray(ff_w, dtype=np.float32)
    ff_b = np.asarray(ff_b, dtype=np.float32)

    # shard over batch; [*, S, V] -> [*, p, k, S] with V = k*128 + p
    def to_pks(x):
        return np.ascontiguousarray(
            x.reshape(NCORES, BL, S, KV, 128).transpose(0, 1, 4, 3, 2))

    x1f = to_pks(output_1)
    x2f = to_pks(output_2)
    x18 = _q8(x1f)
    x1r8 = _q8(x1f - x18.astype(np.float32))
    x28 = _q8(x2f)
    # per-example pack: [p, plane, k, S] contiguous -> one DMA per example
    x_all = np.ascontiguousarray(
        np.stack([x18, x1r8, x28], axis=3)
    ).reshape(NCORES, BL, 128, 3 * KV * S)

    ws = np.ascontiguousarray(
        (mlp_w.T * WSCALE).reshape(KV, 128, H).transpose(1, 0, 2))  # [p,k,H]
    w8 = _q8(ws)
    wr8 = _q8(ws - w8.astype(np.float32))
    w_all = np.ascontiguousarray(
        np.stack([w8, wr8], axis=1)).reshape(128, 2 * KV * H)

    mlpb = np.ascontiguousarray(mlp_b.reshape(MH, 128).T)          # [128, MH]
    mlpb64 = np.ascontiguousarray(mlpb * WSCALE)
    fdw = np.ascontiguousarray(
        fd_w.T.reshape(4, 128, H).transpose(1, 0, 2)
    ).reshape(128, 4 * H).astype(np.float16)
    fdb = np.ascontiguousarray(fd_b.reshape(MH, 128).T)
    ffw = np.ascontiguousarray(
        ff_w.reshape(MH, 128).T).astype(np.float16)                # [128, MH]
    nffb = np.ascontiguousarray(-ff_b.reshape(1, 1))

    in_maps = [
        dict(x_all=x_all[c], w_all=w_all, mlpb=mlpb, mlpb64=mlpb64,
             fdw=fdw, fdb=fdb, ffw=ffw, nffb=nffb)
        for c in range(NCORES)
    ]
    global _LAST_IN_MAPS
    _LAST_IN_MAPS = in_maps
    nc = _get_nc()
    res = bass_utils.run_bass_kernel_spmd(nc, in_maps, core_ids=list(range(NCORES)))
    att = np.concatenate([res.results[c]["out"] for c in range(NCORES)], axis=0)
    return np.ascontiguousarray(att.T)  # [S+1, B]
